# revision 10
# baseline (speedup 1.0000x reference)
"""Causal self-attention Trainium2 kernel (B=2, T=4096, E=768, H=12, D=64).

Sharding: 8 cores = 2 batches x 4 head-groups (3 heads each). Each core:
  - receives x pre-transposed from host (xbT [E, T]) so no PE transposes,
  - computes q/k in [32, 2, t] fp8e4 DoubleRow layout and v in [t, d] fp8e4
    for its 3 heads (projection matmuls stay fp32r),
  - causal attention in S^T layout ([key, query] tiles): S matmuls are
    fp8e4 DoubleRow (d split 32+32), exp on ACT writes fp8e4 directly with
    a -2.0 bias (keeps exp values well inside e4m3 range; cancels in the
    softmax ratio), PV is one DoubleRow matmul per key-block PAIR
    (256-key contraction), denominator via an extra ones-column in v
    (PV row 64 = sum of exp),
  - normalizes via reciprocal + PE broadcast, out-projects with its wo
    row-slice (fp32r) producing a partial y [4096, 768].
Host sums the 4 partials per batch and adds bo.

ATTN_NO_S8 / ATTN_NO_PV8 env flags fall back to the fp32r attention paths.
"""

import os
import sys

sys.path.insert(0, "/opt/trn_rl_repo")

import numpy as np

try:  # persistent jit cache: skips the ~10min neuronxcc compile on re-runs
    import jax

    jax.config.update("jax_compilation_cache_dir", "/tmp/jax_neff_cache")
    jax.config.update("jax_persistent_cache_min_compile_time_secs", 10)
    jax.config.update("jax_persistent_cache_min_entry_size_bytes", 0)
except Exception:
    pass

import concourse.bass as bass
import concourse.mybir as mybir
import concourse.tile as tile
from concourse import bacc
from concourse.bass_utils import run_bass_kernel_spmd

F32 = mybir.dt.float32
F32R = mybir.dt.float32r
FP8 = mybir.dt.float8e4
DR = mybir.MatmulPerfMode.DoubleRow

B, T, E, H = 2, 4096, 768, 12
D = E // H            # 64
HL = 3                # heads per core
CH = HL * D           # 192 channels per core
SB = 512              # query superblock
KB = 128              # key block
NEB = E // 128        # 6 embed tiles
SCALE = 1.0 / np.sqrt(D)
EXP_BIAS = -2.0       # exp(s*SCALE-2): cancels in softmax, keeps fp8 in range

USE_S8 = os.environ.get("ATTN_NO_S8", "") == ""
USE_PV8 = os.environ.get("ATTN_NO_PV8", "") == ""
QKD = FP8 if USE_S8 else F32   # q/k storage dtype
PVD = FP8 if USE_PV8 else F32  # exp(S)/v storage dtype


def _mm(ap):
    # fp32 tiles need the f32r bitcast to be matmul-legal; fp8 is native
    return ap.bitcast(F32R) if ap.dtype == F32 else ap


def build_nc(t_len=T, repeat=1):
    assert t_len % SB == 0
    nsb = t_len // SB       # superblocks
    ntb = t_len // KB       # 128-blocks

    nc = bacc.Bacc("TRN2", target_bir_lowering=False, debug=False, num_devices=8)

    xbT = nc.dram_tensor("xbT", [E, t_len], F32, kind="ExternalInput")
    wqk = nc.dram_tensor("wqk", [E, 2 * CH], F32, kind="ExternalInput")
    wvp = nc.dram_tensor("wvp", [E, 256], F32, kind="ExternalInput")
    wo = nc.dram_tensor("wo", [CH, E], F32, kind="ExternalInput")
    bqk = nc.dram_tensor("bqk", [HL, 2, D], F32, kind="ExternalInput")
    bv = nc.dram_tensor("bv", [CH + HL], F32, kind="ExternalInput")
    cst = nc.dram_tensor("cst", [128, 192], F32, kind="ExternalInput")
    y = nc.dram_tensor("y", [t_len, E], F32, kind="ExternalOutput")

    xbT, wqk, wvp, wo, bqk, bv, cst, y = (
        t.ap() for t in (xbT, wqk, wvp, wo, bqk, bv, cst, y)
    )

    with tile.TileContext(nc) as tc:
        import contextlib

        ctx = contextlib.ExitStack()
        with ctx:
            ctx.enter_context(
                nc.allow_low_precision(reason="fp8/fp32r attention operands")
            )
            const = ctx.enter_context(tc.tile_pool(name="const", bufs=1))
            persist = ctx.enter_context(tc.tile_pool(name="persist", bufs=1))
            xtpool = ctx.enter_context(tc.tile_pool(name="xtpool", bufs=2))
            qspool = ctx.enter_context(tc.tile_pool(name="qspool", bufs=6))
            ospool = ctx.enter_context(tc.tile_pool(name="ospool", bufs=2))
            ptpool = ctx.enter_context(tc.tile_pool(name="ptpool", bufs=4))
            rpool = ctx.enter_context(tc.tile_pool(name="rpool", bufs=2))
            ypool = ctx.enter_context(tc.tile_pool(name="ypool", bufs=2))
            psA = ctx.enter_context(tc.tile_pool(name="psA", bufs=2, space="PSUM"))
            psS = ctx.enter_context(tc.tile_pool(name="psS", bufs=2, space="PSUM"))
            psO = ctx.enter_context(tc.tile_pool(name="psO", bufs=2, space="PSUM"))

            # ---- constants / weights in SBUF ----
            ones65 = const.tile([65, D], F32)
            nc.sync.dma_start(
                out=_mm(ones65[64:65, :]), in_=_mm(cst[64:65, 128 : 128 + D])
            )

            wqk_sb = const.tile([128, NEB, 2 * CH], F32)
            nc.sync.dma_start(
                out=_mm(wqk_sb), in_=_mm(wqk).rearrange("(n p) m -> p n m", p=128)
            )
            wv_sb = const.tile([128, NEB, 256], F32)
            nc.sync.dma_start(
                out=_mm(wv_sb), in_=_mm(wvp).rearrange("(n p) m -> p n m", p=128)
            )
            wo01_sb = const.tile([128, E], F32)
            nc.sync.dma_start(out=_mm(wo01_sb), in_=_mm(wo[0 : 2 * D, :]))
            wo2_sb = const.tile([D, E], F32)
            nc.sync.dma_start(out=_mm(wo2_sb), in_=_mm(wo[2 * D : CH, :]))
            bqk_sb = const.tile([D, HL, 2], F32)
            nc.sync.dma_start(out=bqk_sb, in_=bqk.rearrange("h q p -> p h q"))
            # k-bias copy living at partitions 64..127 (k rows of the packed
            # qk psum) so the staging add is partition-aligned
            bk64_sb = const.tile([128, HL], F32)
            nc.sync.dma_start(
                out=bk64_sb[D : 2 * D, :], in_=bqk[:, 1, :].rearrange("h p -> p h")
            )
            ebias = const.tile([128, 1], F32)
            nc.vector.memset(ebias, float(EXP_BIAS))
            bv_bc = const.tile([128, CH + HL], F32)
            nc.sync.dma_start(
                out=bv_bc,
                in_=bass.AP(
                    tensor=bv.tensor, offset=bv.offset, ap=[[0, 128]] + list(bv.ap)
                ),
            )

            # persistent activations
            if USE_S8:
                kT = [
                    persist.tile([32, 2, t_len], FP8, name=f"kT{h}")
                    for h in range(HL)
                ]
            else:
                kT = [
                    persist.tile([D, t_len], F32, name=f"kT{h}") for h in range(HL)
                ]
            v_sb = persist.tile([128, ntb, HL, D + 1], PVD)

            import contextlib as _cl

            loop_cm = tc.For_i(0, repeat, 1) if repeat > 1 else _cl.nullcontext()

            def phase3(cp, oS):
                # out-projection for t-blocks of superblock cp
                for tb in range(4):
                    tg = cp * 4 + tb
                    y_sb = ypool.tile([128, E], F32, tag="y_sb", name="y_sb")
                    oS01p, oS2p = oS
                    for half in range(2):
                        ps_y = psO.tile([128, 384], F32, tag="psO", name="ps_y")
                        nc.tensor.matmul(
                            ps_y,
                            lhsT=_mm(oS01p[:, tb * KB : (tb + 1) * KB]),
                            rhs=_mm(wo01_sb[:, half * 384 : (half + 1) * 384]),
                            start=True,
                            stop=False,
                        )
                        nc.tensor.matmul(
                            ps_y,
                            lhsT=_mm(oS2p[:, tb * KB : (tb + 1) * KB]),
                            rhs=_mm(wo2_sb[:, half * 384 : (half + 1) * 384]),
                            start=False,
                            stop=True,
                        )
                        nc.vector.tensor_copy(
                            out=y_sb[:, half * 384 : (half + 1) * 384], in_=ps_y
                        )
                    nc.sync.dma_start(
                        out=y[tg * KB : (tg + 1) * KB, :], in_=y_sb
                    )

            with loop_cm:
              for c in range(nsb):
                # ======== phase 1: x^T in, q^T/k^T, v for tokens [c*SB,(c+1)*SB)
                xT = xtpool.tile([128, NEB, SB], F32, tag="xT")
                nc.sync.dma_start(
                    out=_mm(xT),
                    in_=_mm(
                        bass.AP(
                            tensor=xbT.tensor,
                            offset=xbT.offset + c * SB,
                            ap=[[t_len, 128], [128 * t_len, NEB], [1, SB]],
                        )
                    ),
                )
                qS = []
                for h in range(HL):
                    ps_qk = psA.tile([128, SB], F32, tag="psA", name="ps_qk")
                    for eb in range(NEB):
                        nc.tensor.matmul(
                            ps_qk,
                            lhsT=_mm(wqk_sb[:, eb, h * 128 : (h + 1) * 128]),
                            rhs=_mm(xT[:, eb, :]),
                            start=(eb == 0),
                            stop=(eb == NEB - 1),
                        )
                    if USE_S8:
                        qst = qspool.tile([D, SB], FP8, tag="qst", name="qst", bufs=2)
                        nc.vector.tensor_scalar_add(
                            out=qst, in0=ps_qk[0:D, :], scalar1=bqk_sb[:, h, 0:1]
                        )
                        q_h = qspool.tile([32, 2, SB], FP8, tag="qS", name="q_h")
                        nc.sync.dma_start(out=q_h[:, 0, :], in_=qst[0:32, :])
                        nc.sync.dma_start(out=q_h[:, 1, :], in_=qst[32:64, :])
                    else:
                        q_h = qspool.tile([D, SB], F32, tag="qS", name="q_h")
                        nc.vector.tensor_scalar_add(
                            out=_mm(q_h), in0=ps_qk[0:D, :], scalar1=bqk_sb[:, h, 0:1]
                        )
                    qS.append(q_h)
                    # k rows live at psum partitions 64..127. Lane engines
                    # cannot shift partitions, so stage at the same partitions
                    # (adding bias) and let SBUF->SBUF DMAs move them into kT.
                    kst = qspool.tile([128, SB], QKD, tag="kst", name="kst", bufs=2)
                    nc.vector.tensor_scalar_add(
                        out=_mm(kst[D : 2 * D, :]),
                        in0=ps_qk[D : 2 * D, :],
                        scalar1=bk64_sb[D : 2 * D, h : h + 1],
                    )
                    if USE_S8:
                        nc.sync.dma_start(
                            out=kT[h][:, 0, c * SB : (c + 1) * SB],
                            in_=kst[64:96, :],
                        )
                        nc.sync.dma_start(
                            out=kT[h][:, 1, c * SB : (c + 1) * SB],
                            in_=kst[96:128, :],
                        )
                    else:
                        nc.sync.dma_start(
                            out=_mm(kT[h][:, c * SB : (c + 1) * SB]),
                            in_=_mm(kst[D : 2 * D, :]),
                        )
                for tb in range(4):
                    ps_v = psA.tile([128, 256], F32, tag="psA", name="ps_v")
                    for eb in range(NEB):
                        nc.tensor.matmul(
                            ps_v,
                            lhsT=_mm(xT[:, eb, tb * 128 : (tb + 1) * 128]),
                            rhs=_mm(wv_sb[:, eb, :]),
                            start=(eb == 0),
                            stop=(eb == NEB - 1),
                        )
                    nc.vector.tensor_add(
                        out=_mm(v_sb[:, c * 4 + tb, :, 0:D]),
                        in0=ps_v[:, 0:CH].rearrange("p (h d) -> p h d", h=HL),
                        in1=bv_bc[:, 0:CH].rearrange("p (h d) -> p h d", h=HL),
                    )
                    # ones column of v_aug: psum cols CH..CH+2 are x @ 0 = 0,
                    # plus the ones carried in the padded bias
                    nc.vector.tensor_add(
                        out=_mm(v_sb[:, c * 4 + tb, :, D : D + 1]),
                        in0=ps_v[:, CH : CH + HL].rearrange(
                            "p (h o) -> p h o", o=1
                        ),
                        in1=bv_bc[:, CH : CH + HL].rearrange(
                            "p (h o) -> p h o", o=1
                        ),
                    )

                # phase 3 of previous superblock goes here: its inputs (oS)
                # are produced by a DVE chain that lags PE, so slot the
                # already-runnable phase-1 work of this chunk in front of it
                if c > 0:
                    phase3(c - 1, oS_prev)

                # ======== phase 2: attention superblock i=c, all local heads
                nj = 4 * c + 4
                npair = nj // 2
                oS01 = ospool.tile([128, SB], F32, tag="oS01", name="oS01")
                oS2 = ospool.tile([D, SB], F32, tag="oS2", name="oS2")
                oS_prev_local = (oS01, oS2)

                def q0_of(j):
                    # causal: key block j only sees queries >= j*KB - c*SB
                    if j < 4 * c:
                        return 0
                    return min((j - 4 * c) * KB, SB - KB)

                def norm_chain(h, ps_o):
                    # PV(h) -> DVE recip -> PE bcast -> DVE mul
                    recip = rpool.tile([65, SB], F32, tag="recip", name="recip")
                    nc.vector.reciprocal(_mm(recip[64:65, :]), ps_o[D : D + 1, :])
                    # psA slots are idle during attention: use one for the
                    # broadcast so the psS rotation is untouched
                    ps_b = psA.tile([128, SB], F32, tag="psA", name="ps_b")
                    nc.tensor.matmul(
                        ps_b[0:D, :],
                        lhsT=_mm(ones65[64:65, :]),
                        rhs=_mm(recip[64:65, :]),
                        start=True,
                        stop=True,
                    )
                    # walrus: a DVE op may read only ONE non-scalar PSUM
                    # input, so stage the broadcast row in SBUF
                    rb = rpool.tile([D, SB], F32, tag="rbcast", name="rb")
                    nc.vector.tensor_copy(out=rb, in_=ps_b[0:D, :])
                    if h == 0:
                        o_dst = oS01[0:D, :]
                    elif h == 2:
                        o_dst = oS2[:, :]
                    else:
                        o_dst = ospool.tile([D, SB], F32, tag="o1tmp", name="o1tmp")
                    nc.vector.tensor_mul(_mm(o_dst), ps_o[0:D, :], rb)
                    if h == 1:
                        # stack h1 under h0 (partitions 64:128) via DMA, the
                        # only engine that can shift partitions
                        nc.sync.dma_start(
                            out=_mm(oS01[D : 2 * D, :]), in_=_mm(o_dst)
                        )

                def pv_pair(ps_o, h, jp, q00, pt_ap):
                    if USE_PV8:
                        nc.tensor.matmul(
                            ps_o[0 : D + 1, q00:],
                            lhsT=v_sb[:, 2 * jp : 2 * jp + 2, h, :],
                            rhs=pt_ap[:, :, q00:],
                            start=(jp == 0),
                            stop=(jp == npair - 1),
                            perf_mode=DR,
                        )
                    else:
                        for half in range(2):
                            j = 2 * jp + half
                            nc.tensor.matmul(
                                ps_o[0 : D + 1, q00:],
                                lhsT=_mm(v_sb[:, j, h, :]),
                                rhs=_mm(pt_ap[:, half, q00:]),
                                start=(j == 0),
                                stop=(j == nj - 1),
                            )

                def stream(h, depth=2):
                    ps_o = psO.tile([128, SB], F32, tag="psO", name="ps_o")
                    q_ap = qS[h]
                    pend = []
                    for jp in range(npair):
                        j0, j1 = 2 * jp, 2 * jp + 1
                        q00 = q0_of(j0)
                        ps_s2 = psS.tile([128, 2, SB], F32, tag="psS", name="ps_s2")
                        for half, j in ((0, j0), (1, j1)):
                            if USE_S8:
                                nc.tensor.matmul(
                                    ps_s2[:, half, q00:],
                                    lhsT=kT[h][:, :, j * KB : (j + 1) * KB],
                                    rhs=q_ap[:, :, q00:],
                                    start=True,
                                    stop=True,
                                    perf_mode=DR,
                                )
                            else:
                                nc.tensor.matmul(
                                    ps_s2[:, half, q00:],
                                    lhsT=_mm(kT[h][:, j * KB : (j + 1) * KB]),
                                    rhs=_mm(q_ap[:, q00:]),
                                    start=True,
                                    stop=True,
                                )
                        pt2 = ptpool.tile([128, 2, SB], PVD, tag="pt", name="pt2")
                        nc.scalar.activation(
                            out=_mm(pt2[:, :, q00:]),
                            in_=ps_s2[:, :, q00:],
                            func=mybir.ActivationFunctionType.Exp,
                            scale=float(SCALE),
                            bias=ebias,
                        )
                        for half, j in ((0, j0), (1, j1)):
                            if j >= 4 * c:
                                nc.gpsimd.affine_select(
                                    out=_mm(pt2[:, half, q00:]),
                                    in_=_mm(pt2[:, half, q00:]),
                                    compare_op=mybir.AluOpType.is_ge,
                                    fill=0.0,
                                    base=c * SB - j * KB + q00,
                                    pattern=[[1, SB - q00]],
                                    channel_multiplier=-1,
                                )
                        pend.append((jp, q00, pt2))
                        while len(pend) > depth:
                            pv_pair(ps_o, h, *pend.pop(0))
                    for jq in pend:
                        pv_pair(ps_o, h, *jq)
                    return ps_o

                prev = None
                for h in range(HL):
                    ps_o_h = stream(h)
                    if prev is not None:
                        norm_chain(*prev)
                    prev = (h, ps_o_h)
                norm_chain(*prev)
                oS_prev = oS_prev_local
              phase3(nsb - 1, oS_prev)
    nc.compile()
    return nc


def make_in_maps(x, wq, bq, wk, bk, wv, bv, wo, bo, t_len=T):
    x = np.asarray(x, np.float32)
    in_maps = []
    for c in range(8):
        b, g = divmod(c, 4)
        hs = slice(g * CH, (g + 1) * CH)
        wqk_c = np.empty((E, 2 * CH), np.float32)
        bqk_c = np.empty((HL, 2, D), np.float32)
        for hl in range(HL):
            h = g * HL + hl
            wqk_c[:, hl * 128 : hl * 128 + D] = wq[:, h * D : (h + 1) * D]
            wqk_c[:, hl * 128 + D : (hl + 1) * 128] = wk[:, h * D : (h + 1) * D]
            bqk_c[hl, 0] = bq[h * D : (h + 1) * D]
            bqk_c[hl, 1] = bk[h * D : (h + 1) * D]
        wv_c = np.zeros((E, 256), np.float32)
        wv_c[:, :CH] = wv[:, hs]
        bv_c = np.ones(CH + HL, np.float32)
        bv_c[:CH] = np.asarray(bv, np.float32)[hs]
        cst = np.concatenate(
            [np.eye(128, dtype=np.float32), np.ones((128, 64), np.float32)], axis=1
        )
        in_maps.append(
            {
                "xbT": np.ascontiguousarray(x[b, :t_len].T),
                "wqk": wqk_c,
                "wvp": wv_c,
                "wo": np.ascontiguousarray(np.asarray(wo, np.float32)[hs]),
                "bqk": bqk_c,
                "bv": bv_c,
                "cst": cst,
            }
        )
    return in_maps


_NC_CACHE = {}


def get_nc(t_len=T):
    if t_len not in _NC_CACHE:
        _NC_CACHE[t_len] = build_nc(t_len)
    return _NC_CACHE[t_len]


def _build_sharded_nodonate(nc, n_cores=8):
    """Mirror bass2jax.run_bass_via_pjrt's multi-core path, minus donation,
    returning (jitted_fn, in_names, out_names, out_avals). Without donation a
    call can be repeated on device-resident arrays for timing. Safe here: the
    kernel writes every element of y."""
    import jax
    from jax.sharding import Mesh, PartitionSpec
    from jax.experimental.shard_map import shard_map

    from concourse import bass2jax
    from concourse.bass2jax import _bass_exec_p

    bass2jax.install_neuronx_cc_hook()
    part_name = nc.partition_id_tensor.name if nc.partition_id_tensor else None

    in_names, out_names, out_avals = [], [], []
    for alloc in nc.m.functions[0].allocations:
        if not isinstance(alloc, mybir.MemoryLocationSet):
            continue
        name = alloc.memorylocations[0].name
        if alloc.kind == "ExternalInput":
            if name != part_name:
                in_names.append(name)
        elif alloc.kind == "ExternalOutput":
            shape = tuple(alloc.tensor_shape)
            dtype = mybir.dt.np(alloc.dtype)
            out_names.append(name)
            out_avals.append(jax.core.ShapedArray(shape, dtype))
    n_params = len(in_names)
    all_names = in_names + out_names
    if part_name is not None:
        all_names = all_names + [part_name]

    def _body(*args):
        operands = list(args)
        if part_name is not None:
            operands.append(bass2jax.partition_id_tensor())
        outs = _bass_exec_p.bind(
            *operands,
            out_avals=tuple(out_avals),
            in_names=tuple(all_names),
            out_names=tuple(out_names),
            lowering_input_output_aliases=(),
            sim_require_finite=True,
            sim_require_nnan=True,
            nc=nc,
        )
        return tuple(outs)

    devices = jax.devices()[:n_cores]
    mesh = Mesh(np.asarray(devices), ("core",))
    n_out = len(out_names)
    sharded = jax.jit(
        shard_map(
            _body,
            mesh=mesh,
            in_specs=(PartitionSpec("core"),) * (n_params + n_out),
            out_specs=(PartitionSpec("core"),) * n_out,
            check_rep=False,
        ),
        keep_unused=True,
    )
    return sharded, in_names, out_names, out_avals


def run_timed(nc, in_maps, iters=20):
    """Execute on HW repeatedly with device-resident args; returns
    (per-core results, sorted per-call walls in seconds)."""
    import time

    import jax

    n_cores = len(in_maps)
    sharded, in_names, out_names, out_avals = _build_sharded_nodonate(nc, n_cores)
    concat_in = [
        np.concatenate([np.asarray(m[name]) for m in in_maps], axis=0)
        for name in in_names
    ]
    concat_zero = [
        np.zeros((n_cores * a.shape[0], *a.shape[1:]), a.dtype) for a in out_avals
    ]
    args = [jax.device_put(a) for a in concat_in + concat_zero]
    out = sharded(*args)  # compile + first run
    jax.block_until_ready(out)
    walls = []
    for _ in range(iters):
        t0 = time.perf_counter()
        out2 = sharded(*args)
        jax.block_until_ready(out2)
        walls.append(time.perf_counter() - t0)
    results = [
        {
            name: np.asarray(out[i]).reshape(n_cores, *out_avals[i].shape)[c]
            for i, name in enumerate(out_names)
        }
        for c in range(n_cores)
    ]
    return results, sorted(walls)


def baseline_rtt(iters=20):
    """Axon dispatch floor: same path with a trivial 8-core kernel."""
    nc = bacc.Bacc("TRN2", target_bir_lowering=False, debug=False, num_devices=8)
    a = nc.dram_tensor("a", [128, 128], F32, kind="ExternalInput")
    b = nc.dram_tensor("b", [128, 128], F32, kind="ExternalOutput")
    a, b = a.ap(), b.ap()
    with tile.TileContext(nc) as tc:
        with tc.tile_pool(name="p", bufs=1) as p:
            t = p.tile([128, 128], F32)
            nc.sync.dma_start(out=t, in_=a)
            nc.scalar.mul(out=t, in_=t, mul=2.0)
            nc.sync.dma_start(out=b, in_=t)
    nc.compile()
    in_maps = [{"a": np.zeros((128, 128), np.float32)} for _ in range(8)]
    _, walls = run_timed(nc, in_maps, iters=iters)
    return walls


def kernel(x, wq, bq, wk, bk, wv, bv, wo, bo, _trace=False, _trace_kwargs=None):
    nc = get_nc()
    in_maps = make_in_maps(x, wq, bq, wk, bk, wv, bv, wo, bo)
    res = run_bass_kernel_spmd(
        nc, in_maps, list(range(8)), trace=_trace, **(_trace_kwargs or {})
    )
    bo = np.asarray(bo, np.float32)
    out = np.empty((B, T, E), np.float32)
    for b in range(B):
        acc = res.results[b * 4]["y"].astype(np.float32).copy()
        for g in range(1, 4):
            acc += res.results[b * 4 + g]["y"]
        out[b] = acc + bo
    if _trace:
        return out, res
    return out


# revision 15
# speedup vs baseline: 1.2175x; 1.2175x over previous
"""Causal self-attention Trainium2 kernel (B=2, T=4096, E=768, H=12, D=64).

Sharding: 8 cores = 2 batches x 4 head-groups (3 heads each). Each core:
  - receives x pre-transposed from host (xbT [E, T]) so no PE transposes,
  - computes q/k in transposed layout [d, t] and v in natural layout [t, d]
    for its 3 heads (fp32r matmuls),
  - causal attention in S^T layout ([key, query] tiles). All attention
    matmul contractions are padded to K=128 (kT rows 64..127 are zero);
    K=64 matmuls run ~2.7x slower on HW (measured) than K=128,
  - exp on ACT per key-block pair, denominator via an extra ones-column
    appended to v (PV matmul row 64 = sum of exp),
  - normalizes via reciprocal + PE broadcast, out-projects with its wo
    row-slice (zero-padded to 128 rows for the second K slice) producing a
    partial y [4096, 768].
Host sums the 4 partials per batch and adds bo.
"""

import os
import sys

sys.path.insert(0, "/opt/trn_rl_repo")

import numpy as np

try:  # persistent jit cache: skips the ~10min neuronxcc compile on re-runs
    import jax

    jax.config.update("jax_compilation_cache_dir", "/tmp/jax_neff_cache")
    jax.config.update("jax_persistent_cache_min_compile_time_secs", 10)
    jax.config.update("jax_persistent_cache_min_entry_size_bytes", 0)
except Exception:
    pass

import concourse.bass as bass
import concourse.mybir as mybir
import concourse.tile as tile
from concourse import bacc
from concourse.bass_utils import run_bass_kernel_spmd

F32 = mybir.dt.float32
F32R = mybir.dt.float32r

B, T, E, H = 2, 4096, 768, 12
D = E // H            # 64
HL = 3                # heads per core
CH = HL * D           # 192 channels per core
SB = 512              # query superblock
KB = 128              # key block
NEB = E // 128        # 6 embed tiles
SCALE = 1.0 / np.sqrt(D)


def _mm(ap):
    return ap.bitcast(F32R) if ap.dtype == F32 else ap


def build_nc(t_len=T, repeat=1):
    assert t_len % SB == 0
    nsb = t_len // SB       # superblocks
    ntb = t_len // KB       # 128-blocks

    nc = bacc.Bacc("TRN2", target_bir_lowering=False, debug=False, num_devices=8)

    xbT = nc.dram_tensor("xbT", [E, t_len], F32, kind="ExternalInput")
    wqk = nc.dram_tensor("wqk", [E, 2 * CH], F32, kind="ExternalInput")
    wvp = nc.dram_tensor("wvp", [E, 256], F32, kind="ExternalInput")
    wo = nc.dram_tensor("wo", [256, E], F32, kind="ExternalInput")
    bqk = nc.dram_tensor("bqk", [HL, 2, D], F32, kind="ExternalInput")
    bv = nc.dram_tensor("bv", [CH + HL], F32, kind="ExternalInput")
    cst = nc.dram_tensor("cst", [128, 192], F32, kind="ExternalInput")
    y = nc.dram_tensor("y", [t_len, E], F32, kind="ExternalOutput")

    xbT, wqk, wvp, wo, bqk, bv, cst, y = (
        t.ap() for t in (xbT, wqk, wvp, wo, bqk, bv, cst, y)
    )

    with tile.TileContext(nc) as tc:
        import contextlib

        ctx = contextlib.ExitStack()
        with ctx:
            ctx.enter_context(
                nc.allow_low_precision(reason="fp32r rounding of matmul operands")
            )
            const = ctx.enter_context(tc.tile_pool(name="const", bufs=1))
            persist = ctx.enter_context(tc.tile_pool(name="persist", bufs=1))
            xtpool = ctx.enter_context(tc.tile_pool(name="xtpool", bufs=2))
            qspool = ctx.enter_context(tc.tile_pool(name="qspool", bufs=6))
            ospool = ctx.enter_context(tc.tile_pool(name="ospool", bufs=2))
            ptpool = ctx.enter_context(tc.tile_pool(name="ptpool", bufs=3))
            rpool = ctx.enter_context(tc.tile_pool(name="rpool", bufs=2))
            ypool = ctx.enter_context(tc.tile_pool(name="ypool", bufs=2))
            psA = ctx.enter_context(tc.tile_pool(name="psA", bufs=2, space="PSUM"))
            psS = ctx.enter_context(tc.tile_pool(name="psS", bufs=2, space="PSUM"))
            psO = ctx.enter_context(tc.tile_pool(name="psO", bufs=2, space="PSUM"))

            # ---- constants / weights in SBUF ----
            ones65 = const.tile([65, D], F32)
            nc.sync.dma_start(
                out=_mm(ones65[64:65, :]), in_=_mm(cst[64:65, 128 : 128 + D])
            )

            wqk_sb = const.tile([128, NEB, 2 * CH], F32)
            nc.sync.dma_start(
                out=_mm(wqk_sb), in_=_mm(wqk).rearrange("(n p) m -> p n m", p=128)
            )
            wv_sb = const.tile([128, NEB, 256], F32)
            nc.sync.dma_start(
                out=_mm(wv_sb), in_=_mm(wvp).rearrange("(n p) m -> p n m", p=128)
            )
            wo01_sb = const.tile([128, E], F32)
            nc.sync.dma_start(out=_mm(wo01_sb), in_=_mm(wo[0:128, :]))
            # rows 64..127 of the second K slice are zero (host-padded)
            wo2_sb = const.tile([128, E], F32)
            nc.sync.dma_start(out=_mm(wo2_sb), in_=_mm(wo[128:256, :]))
            bqk_sb = const.tile([D, HL, 2], F32)
            nc.sync.dma_start(out=bqk_sb, in_=bqk.rearrange("h q p -> p h q"))
            # k-bias copy living at partitions 64..127 (k rows of the packed
            # qk psum) so the staging add is partition-aligned
            bk64_sb = const.tile([128, HL], F32)
            nc.sync.dma_start(
                out=bk64_sb[D : 2 * D, :], in_=bqk[:, 1, :].rearrange("h p -> p h")
            )
            bv_bc = const.tile([128, CH + HL], F32)
            nc.sync.dma_start(
                out=bv_bc,
                in_=bass.AP(
                    tensor=bv.tensor, offset=bv.offset, ap=[[0, 128]] + list(bv.ap)
                ),
            )

            # persistent activations. kT rows 64..127 stay zero forever: the
            # K=128 S-matmul contraction ignores whatever sits in the padded
            # q rows (0 * garbage), but the pad rows themselves must be 0.
            kT = [persist.tile([128, t_len], F32, name=f"kT{h}") for h in range(HL)]

            def dma_zero(dst, parts, cols):
                # cst cols 0..127 are zeros (host); stride-0 repeat covers cols
                assert cols % 128 == 0
                nc.sync.dma_start(
                    out=_mm(dst),
                    in_=_mm(
                        bass.AP(
                            tensor=cst.tensor,
                            offset=cst.offset + D * 192,
                            ap=[[192, parts], [0, cols // 128], [1, 128]],
                        )
                    ),
                )

            for h in range(HL):
                dma_zero(kT[h][D:128, :], D, t_len)
            v_sb = persist.tile([128, ntb, HL, D + 1], F32)

            import contextlib as _cl

            loop_cm = tc.For_i(0, repeat, 1) if repeat > 1 else _cl.nullcontext()

            def phase3(cp, oS):
                # out-projection for t-blocks of superblock cp
                for tb in range(4):
                    tg = cp * 4 + tb
                    y_sb = ypool.tile([128, E], F32, tag="y_sb", name="y_sb")
                    oS01p, oS2p = oS
                    for half in range(2):
                        ps_y = psO.tile([128, 384], F32, tag="psO", name="ps_y")
                        nc.tensor.matmul(
                            ps_y,
                            lhsT=_mm(oS01p[:, tb * KB : (tb + 1) * KB]),
                            rhs=_mm(wo01_sb[:, half * 384 : (half + 1) * 384]),
                            start=True,
                            stop=False,
                        )
                        nc.tensor.matmul(
                            ps_y,
                            lhsT=_mm(oS2p[:, tb * KB : (tb + 1) * KB]),
                            rhs=_mm(wo2_sb[:, half * 384 : (half + 1) * 384]),
                            start=False,
                            stop=True,
                        )
                        nc.vector.tensor_copy(
                            out=y_sb[:, half * 384 : (half + 1) * 384], in_=ps_y
                        )
                    nc.sync.dma_start(
                        out=y[tg * KB : (tg + 1) * KB, :], in_=y_sb
                    )

            with loop_cm:
              for c in range(nsb):
                # ======== phase 1: x^T in, q^T/k^T, v for tokens [c*SB,(c+1)*SB)
                xT = xtpool.tile([128, NEB, SB], F32, tag="xT")
                nc.sync.dma_start(
                    out=_mm(xT),
                    in_=_mm(
                        bass.AP(
                            tensor=xbT.tensor,
                            offset=xbT.offset + c * SB,
                            ap=[[t_len, 128], [128 * t_len, NEB], [1, SB]],
                        )
                    ),
                )
                qS = []
                for h in range(HL):
                    ps_qk = psA.tile([128, SB], F32, tag="psA", name="ps_qk")
                    for eb in range(NEB):
                        nc.tensor.matmul(
                            ps_qk,
                            lhsT=_mm(wqk_sb[:, eb, h * 128 : (h + 1) * 128]),
                            rhs=_mm(xT[:, eb, :]),
                            start=(eb == 0),
                            stop=(eb == NEB - 1),
                        )
                    # q padded to 128 partitions: rows 64.. are multiplied by
                    # the zero rows of kT, but must not contain inf/nan
                    q_h = qspool.tile([128, SB], F32, tag="qS", name="q_h")
                    dma_zero(q_h[D:128, :], D, SB)
                    nc.vector.tensor_scalar_add(
                        out=_mm(q_h[0:D, :]),
                        in0=ps_qk[0:D, :],
                        scalar1=bqk_sb[:, h, 0:1],
                    )
                    qS.append(q_h)
                    # k rows live at psum partitions 64..127. Lane engines
                    # cannot shift partitions, so stage at the same partitions
                    # (adding bias) and let an SBUF->SBUF DMA move them to
                    # partition base 0 in kT.
                    kst = qspool.tile([128, SB], F32, tag="kst", name="kst", bufs=2)
                    nc.vector.tensor_scalar_add(
                        out=_mm(kst[D : 2 * D, :]),
                        in0=ps_qk[D : 2 * D, :],
                        scalar1=bk64_sb[D : 2 * D, h : h + 1],
                    )
                    nc.sync.dma_start(
                        out=_mm(kT[h][0:D, c * SB : (c + 1) * SB]),
                        in_=_mm(kst[D : 2 * D, :]),
                    )
                for tb in range(4):
                    ps_v = psA.tile([128, 256], F32, tag="psA", name="ps_v")
                    for eb in range(NEB):
                        nc.tensor.matmul(
                            ps_v,
                            lhsT=_mm(xT[:, eb, tb * 128 : (tb + 1) * 128]),
                            rhs=_mm(wv_sb[:, eb, :]),
                            start=(eb == 0),
                            stop=(eb == NEB - 1),
                        )
                    nc.vector.tensor_add(
                        out=_mm(v_sb[:, c * 4 + tb, :, 0:D]),
                        in0=ps_v[:, 0:CH].rearrange("p (h d) -> p h d", h=HL),
                        in1=bv_bc[:, 0:CH].rearrange("p (h d) -> p h d", h=HL),
                    )
                    # ones column of v_aug: psum cols CH..CH+2 are x @ 0 = 0,
                    # plus the ones carried in the padded bias
                    nc.vector.tensor_add(
                        out=_mm(v_sb[:, c * 4 + tb, :, D : D + 1]),
                        in0=ps_v[:, CH : CH + HL].rearrange(
                            "p (h o) -> p h o", o=1
                        ),
                        in1=bv_bc[:, CH : CH + HL].rearrange(
                            "p (h o) -> p h o", o=1
                        ),
                    )

                # phase 3 of previous superblock goes here: its inputs (oS)
                # are produced by a DVE chain that lags PE, so slot the
                # already-runnable phase-1 work of this chunk in front of it
                if c > 0:
                    phase3(c - 1, oS_prev)

                # ======== phase 2: attention superblock i=c, all local heads
                nj = 4 * c + 4
                npair = nj // 2
                oS01 = ospool.tile([128, SB], F32, tag="oS01", name="oS01")
                oS2 = ospool.tile([128, SB], F32, tag="oS2", name="oS2")
                dma_zero(oS2[D:128, :], D, SB)
                oS_prev_local = (oS01, oS2)

                def q0_of(j):
                    # causal slice: key block j only sees queries
                    # >= j*KB - c*SB; keep the moving dim >= 256 so
                    # fp32r stays at full rate
                    if j < 4 * c:
                        return 0
                    return min((j - 4 * c) * KB, SB - 256)

                def norm_chain(h, ps_o):
                    # PV(h) -> DVE recip -> PE bcast -> DVE mul
                    recip = rpool.tile([65, SB], F32, tag="recip", name="recip")
                    nc.vector.reciprocal(_mm(recip[64:65, :]), ps_o[D : D + 1, :])
                    # psA slots are idle during attention: use one for the
                    # broadcast so the psS rotation is untouched
                    ps_b = psA.tile([128, SB], F32, tag="psA", name="ps_b")
                    nc.tensor.matmul(
                        ps_b[0:D, :],
                        lhsT=_mm(ones65[64:65, :]),
                        rhs=_mm(recip[64:65, :]),
                        start=True,
                        stop=True,
                    )
                    # walrus: a DVE op may read only ONE non-scalar PSUM
                    # input, so stage the broadcast row in SBUF
                    rb = rpool.tile([D, SB], F32, tag="rbcast", name="rb")
                    nc.vector.tensor_copy(out=rb, in_=ps_b[0:D, :])
                    if h == 0:
                        o_dst = oS01[0:D, :]
                    elif h == 2:
                        o_dst = oS2[0:D, :]
                    else:
                        o_dst = ospool.tile([D, SB], F32, tag="o1tmp", name="o1tmp")
                    nc.vector.tensor_mul(_mm(o_dst), ps_o[0:D, :], rb)
                    if h == 1:
                        # stack h1 under h0 (partitions 64:128) via DMA, the
                        # only engine that can shift partitions
                        nc.sync.dma_start(
                            out=_mm(oS01[D : 2 * D, :]), in_=_mm(o_dst)
                        )

                def stream(h, depth=2):
                    ps_o = psO.tile([128, SB], F32, tag="psO", name="ps_o")
                    q_ap = qS[h]
                    pend = []

                    def pv_step(j, q0, pt_ap):
                        nc.tensor.matmul(
                            ps_o[0 : D + 1, q0:],
                            lhsT=_mm(v_sb[:, j, h, :]),
                            rhs=_mm(pt_ap[:, q0:]),
                            start=(j == 0),
                            stop=(j == nj - 1),
                        )

                    for jp in range(npair):
                        j0, j1 = 2 * jp, 2 * jp + 1
                        q00 = q0_of(j0)
                        ps_s2 = psS.tile([128, 2, SB], F32, tag="psS", name="ps_s2")
                        for half, j in ((0, j0), (1, j1)):
                            nc.tensor.matmul(
                                ps_s2[:, half, q00:],
                                lhsT=_mm(kT[h][:, j * KB : (j + 1) * KB]),
                                rhs=_mm(q_ap[:, q00:]),
                                start=True,
                                stop=True,
                            )
                        pt2 = ptpool.tile([128, 2, SB], F32, tag="pt", name="pt2")
                        nc.scalar.activation(
                            out=_mm(pt2[:, :, q00:]),
                            in_=ps_s2[:, :, q00:],
                            func=mybir.ActivationFunctionType.Exp,
                            scale=float(SCALE),
                        )
                        for half, j in ((0, j0), (1, j1)):
                            if j >= 4 * c:
                                nc.gpsimd.affine_select(
                                    out=_mm(pt2[:, half, q00:]),
                                    in_=_mm(pt2[:, half, q00:]),
                                    compare_op=mybir.AluOpType.is_ge,
                                    fill=0.0,
                                    base=c * SB - j * KB + q00,
                                    pattern=[[1, SB - q00]],
                                    channel_multiplier=-1,
                                )
                        pend.append((j0, q00, pt2[:, 0, :]))
                        pend.append((j1, q00, pt2[:, 1, :]))
                        while len(pend) > 2 * depth:
                            pv_step(*pend.pop(0))
                    for jq in pend:
                        pv_step(*jq)
                    return ps_o

                prev = None
                for h in range(HL):
                    ps_o_h = stream(h)
                    if prev is not None:
                        norm_chain(*prev)
                    prev = (h, ps_o_h)
                norm_chain(*prev)
                oS_prev = oS_prev_local
              phase3(nsb - 1, oS_prev)
    nc.compile()
    return nc


def make_in_maps(x, wq, bq, wk, bk, wv, bv, wo, bo, t_len=T):
    x = np.asarray(x, np.float32)
    in_maps = []
    for c in range(8):
        b, g = divmod(c, 4)
        hs = slice(g * CH, (g + 1) * CH)
        wqk_c = np.empty((E, 2 * CH), np.float32)
        bqk_c = np.empty((HL, 2, D), np.float32)
        for hl in range(HL):
            h = g * HL + hl
            wqk_c[:, hl * 128 : hl * 128 + D] = wq[:, h * D : (h + 1) * D]
            wqk_c[:, hl * 128 + D : (hl + 1) * 128] = wk[:, h * D : (h + 1) * D]
            bqk_c[hl, 0] = bq[h * D : (h + 1) * D]
            bqk_c[hl, 1] = bk[h * D : (h + 1) * D]
        wv_c = np.zeros((E, 256), np.float32)
        wv_c[:, :CH] = wv[:, hs]
        bv_c = np.ones(CH + HL, np.float32)
        bv_c[:CH] = np.asarray(bv, np.float32)[hs]
        wo_c = np.zeros((256, E), np.float32)
        wo_c[:CH] = np.asarray(wo, np.float32)[hs]
        # cols 0..127: zeros (device-side zero fills); cols 128..191: ones
        cst = np.concatenate(
            [np.zeros((128, 128), np.float32), np.ones((128, 64), np.float32)],
            axis=1,
        )
        in_maps.append(
            {
                "xbT": np.ascontiguousarray(x[b, :t_len].T),
                "wqk": wqk_c,
                "wvp": wv_c,
                "wo": wo_c,
                "bqk": bqk_c,
                "bv": bv_c,
                "cst": cst,
            }
        )
    return in_maps


_NC_CACHE = {}


def get_nc(t_len=T):
    if t_len not in _NC_CACHE:
        _NC_CACHE[t_len] = build_nc(t_len)
    return _NC_CACHE[t_len]


def _build_sharded_nodonate(nc, n_cores=8):
    """Mirror bass2jax.run_bass_via_pjrt's multi-core path, minus donation,
    returning (jitted_fn, in_names, out_names, out_avals). Without donation a
    call can be repeated on device-resident arrays for timing. Safe here: the
    kernel writes every element of y."""
    import jax
    from jax.sharding import Mesh, PartitionSpec
    from jax.experimental.shard_map import shard_map

    from concourse import bass2jax
    from concourse.bass2jax import _bass_exec_p

    bass2jax.install_neuronx_cc_hook()
    part_name = nc.partition_id_tensor.name if nc.partition_id_tensor else None

    in_names, out_names, out_avals = [], [], []
    for alloc in nc.m.functions[0].allocations:
        if not isinstance(alloc, mybir.MemoryLocationSet):
            continue
        name = alloc.memorylocations[0].name
        if alloc.kind == "ExternalInput":
            if name != part_name:
                in_names.append(name)
        elif alloc.kind == "ExternalOutput":
            shape = tuple(alloc.tensor_shape)
            dtype = mybir.dt.np(alloc.dtype)
            out_names.append(name)
            out_avals.append(jax.core.ShapedArray(shape, dtype))
    n_params = len(in_names)
    all_names = in_names + out_names
    if part_name is not None:
        all_names = all_names + [part_name]

    def _body(*args):
        operands = list(args)
        if part_name is not None:
            operands.append(bass2jax.partition_id_tensor())
        outs = _bass_exec_p.bind(
            *operands,
            out_avals=tuple(out_avals),
            in_names=tuple(all_names),
            out_names=tuple(out_names),
            lowering_input_output_aliases=(),
            sim_require_finite=True,
            sim_require_nnan=True,
            nc=nc,
        )
        return tuple(outs)

    devices = jax.devices()[:n_cores]
    mesh = Mesh(np.asarray(devices), ("core",))
    n_out = len(out_names)
    sharded = jax.jit(
        shard_map(
            _body,
            mesh=mesh,
            in_specs=(PartitionSpec("core"),) * (n_params + n_out),
            out_specs=(PartitionSpec("core"),) * n_out,
            check_rep=False,
        ),
        keep_unused=True,
    )
    return sharded, in_names, out_names, out_avals


def run_timed(nc, in_maps, iters=20):
    """Execute on HW repeatedly with device-resident args; returns
    (per-core results, sorted per-call walls in seconds)."""
    import time

    import jax

    n_cores = len(in_maps)
    sharded, in_names, out_names, out_avals = _build_sharded_nodonate(nc, n_cores)
    concat_in = [
        np.concatenate([np.asarray(m[name]) for m in in_maps], axis=0)
        for name in in_names
    ]
    concat_zero = [
        np.zeros((n_cores * a.shape[0], *a.shape[1:]), a.dtype) for a in out_avals
    ]
    args = [jax.device_put(a) for a in concat_in + concat_zero]
    out = sharded(*args)  # compile + first run
    jax.block_until_ready(out)
    walls = []
    for _ in range(iters):
        t0 = time.perf_counter()
        out2 = sharded(*args)
        jax.block_until_ready(out2)
        walls.append(time.perf_counter() - t0)
    results = [
        {
            name: np.asarray(out[i]).reshape(n_cores, *out_avals[i].shape)[c]
            for i, name in enumerate(out_names)
        }
        for c in range(n_cores)
    ]
    return results, sorted(walls)


def baseline_rtt(iters=20):
    """Axon dispatch floor: same path with a trivial 8-core kernel."""
    nc = bacc.Bacc("TRN2", target_bir_lowering=False, debug=False, num_devices=8)
    a = nc.dram_tensor("a", [128, 128], F32, kind="ExternalInput")
    b = nc.dram_tensor("b", [128, 128], F32, kind="ExternalOutput")
    a, b = a.ap(), b.ap()
    with tile.TileContext(nc) as tc:
        with tc.tile_pool(name="p", bufs=1) as p:
            t = p.tile([128, 128], F32)
            nc.sync.dma_start(out=t, in_=a)
            nc.scalar.mul(out=t, in_=t, mul=2.0)
            nc.sync.dma_start(out=b, in_=t)
    nc.compile()
    in_maps = [{"a": np.zeros((128, 128), np.float32)} for _ in range(8)]
    _, walls = run_timed(nc, in_maps, iters=iters)
    return walls


def kernel(x, wq, bq, wk, bk, wv, bv, wo, bo, _trace=False, _trace_kwargs=None):
    nc = get_nc()
    in_maps = make_in_maps(x, wq, bq, wk, bk, wv, bv, wo, bo)
    res = run_bass_kernel_spmd(
        nc, in_maps, list(range(8)), trace=_trace, **(_trace_kwargs or {})
    )
    bo = np.asarray(bo, np.float32)
    out = np.empty((B, T, E), np.float32)
    for b in range(B):
        acc = res.results[b * 4]["y"].astype(np.float32).copy()
        for g in range(1, 4):
            acc += res.results[b * 4 + g]["y"]
        out[b] = acc + bo
    if _trace:
        return out, res
    return out


# revision 16
# speedup vs baseline: 1.3146x; 1.0797x over previous
"""Causal self-attention Trainium2 kernel (B=2, T=4096, E=768, H=12, D=64).

Sharding: 8 cores = 2 batches x 4 head-groups (3 heads each). Each core:
  - receives x pre-transposed from host (xbT [E, T]) so no PE transposes,
  - computes q/k in transposed layout [d, t] and v in natural layout [t, d]
    for its 3 heads (fp32r matmuls),
  - causal attention in S^T layout ([key, query] tiles). All attention
    matmul contractions are padded to K=128 (kT rows 64..127 are zero);
    K=64 matmuls run ~2.7x slower on HW (measured) than K=128,
  - exp on ACT per key-block pair, denominator via an extra ones-column
    appended to v (PV matmul row 64 = sum of exp),
  - normalizes via reciprocal + PE broadcast, out-projects with its wo
    row-slice (zero-padded to 128 rows for the second K slice) producing a
    partial y [4096, 768].
Host sums the 4 partials per batch and adds bo.
"""

import os
import sys

sys.path.insert(0, "/opt/trn_rl_repo")

import numpy as np

try:  # persistent jit cache: skips the ~10min neuronxcc compile on re-runs
    import jax

    jax.config.update("jax_compilation_cache_dir", "/tmp/jax_neff_cache")
    jax.config.update("jax_persistent_cache_min_compile_time_secs", 10)
    jax.config.update("jax_persistent_cache_min_entry_size_bytes", 0)
except Exception:
    pass

import concourse.bass as bass
import concourse.mybir as mybir
import concourse.tile as tile
from concourse import bacc
from concourse.bass_utils import run_bass_kernel_spmd

F32 = mybir.dt.float32
F32R = mybir.dt.float32r

B, T, E, H = 2, 4096, 768, 12
D = E // H            # 64
HL = 3                # heads per core
CH = HL * D           # 192 channels per core
SB = 512              # query superblock
KB = 128              # key block
NEB = E // 128        # 6 embed tiles
SCALE = 1.0 / np.sqrt(D)


def _mm(ap):
    return ap.bitcast(F32R) if ap.dtype == F32 else ap


def build_nc(t_len=T, repeat=1):
    assert t_len % SB == 0
    nsb = t_len // SB       # superblocks
    ntb = t_len // KB       # 128-blocks

    nc = bacc.Bacc("TRN2", target_bir_lowering=False, debug=False, num_devices=8)

    xbT = nc.dram_tensor("xbT", [E, t_len], F32, kind="ExternalInput")
    wqk = nc.dram_tensor("wqk", [E, 2 * CH], F32, kind="ExternalInput")
    wvp = nc.dram_tensor("wvp", [E, 256], F32, kind="ExternalInput")
    wo = nc.dram_tensor("wo", [256, E], F32, kind="ExternalInput")
    bqk = nc.dram_tensor("bqk", [HL, 2, D], F32, kind="ExternalInput")
    bv = nc.dram_tensor("bv", [CH + HL], F32, kind="ExternalInput")
    cst = nc.dram_tensor("cst", [128, 192], F32, kind="ExternalInput")
    y = nc.dram_tensor("y", [t_len, E], F32, kind="ExternalOutput")

    xbT, wqk, wvp, wo, bqk, bv, cst, y = (
        t.ap() for t in (xbT, wqk, wvp, wo, bqk, bv, cst, y)
    )

    with tile.TileContext(nc) as tc:
        import contextlib

        ctx = contextlib.ExitStack()
        with ctx:
            ctx.enter_context(
                nc.allow_low_precision(reason="fp32r rounding of matmul operands")
            )
            const = ctx.enter_context(tc.tile_pool(name="const", bufs=1))
            persist = ctx.enter_context(tc.tile_pool(name="persist", bufs=1))
            xtpool = ctx.enter_context(tc.tile_pool(name="xtpool", bufs=2))
            qspool = ctx.enter_context(tc.tile_pool(name="qspool", bufs=6))
            ospool = ctx.enter_context(tc.tile_pool(name="ospool", bufs=2))
            ptpool = ctx.enter_context(tc.tile_pool(name="ptpool", bufs=3))
            rpool = ctx.enter_context(tc.tile_pool(name="rpool", bufs=2))
            ypool = ctx.enter_context(tc.tile_pool(name="ypool", bufs=2))
            psA = ctx.enter_context(tc.tile_pool(name="psA", bufs=2, space="PSUM"))
            psS = ctx.enter_context(tc.tile_pool(name="psS", bufs=2, space="PSUM"))
            psO = ctx.enter_context(tc.tile_pool(name="psO", bufs=2, space="PSUM"))

            # ---- constants / weights in SBUF ----
            ones65 = const.tile([65, D], F32)
            nc.sync.dma_start(
                out=_mm(ones65[64:65, :]), in_=_mm(cst[64:65, 128 : 128 + D])
            )

            wqk_sb = const.tile([128, NEB, 2 * CH], F32)
            nc.sync.dma_start(
                out=_mm(wqk_sb), in_=_mm(wqk).rearrange("(n p) m -> p n m", p=128)
            )
            wv_sb = const.tile([128, NEB, 256], F32)
            nc.sync.dma_start(
                out=_mm(wv_sb), in_=_mm(wvp).rearrange("(n p) m -> p n m", p=128)
            )
            wo01_sb = const.tile([128, E], F32)
            nc.sync.dma_start(out=_mm(wo01_sb), in_=_mm(wo[0:128, :]))
            # rows 64..127 of the second K slice are zero (host-padded)
            wo2_sb = const.tile([128, E], F32)
            nc.sync.dma_start(out=_mm(wo2_sb), in_=_mm(wo[128:256, :]))
            bqk_sb = const.tile([D, HL, 2], F32)
            nc.sync.dma_start(out=bqk_sb, in_=bqk.rearrange("h q p -> p h q"))
            # k-bias copy living at partitions 64..127 (k rows of the packed
            # qk psum) so the staging add is partition-aligned
            bk64_sb = const.tile([128, HL], F32)
            nc.sync.dma_start(
                out=bk64_sb[D : 2 * D, :], in_=bqk[:, 1, :].rearrange("h p -> p h")
            )
            bv_bc = const.tile([128, CH + HL], F32)
            nc.sync.dma_start(
                out=bv_bc,
                in_=bass.AP(
                    tensor=bv.tensor, offset=bv.offset, ap=[[0, 128]] + list(bv.ap)
                ),
            )

            # persistent activations, split per superblock chunk so the
            # interleaved phase-1 writes of chunk c+1 never alias the
            # attention reads of chunk c. kT rows 64..127 stay zero: the
            # K=128 S-matmul contraction is padded (K=64 matmuls run ~2.7x
            # slower on HW), and the padded q rows must multiply zeros.
            def dma_zero(dst, parts, cols):
                # cst cols 0..127 are zeros (host); stride-0 repeat covers cols
                assert cols % 128 == 0
                nc.sync.dma_start(
                    out=_mm(dst),
                    in_=_mm(
                        bass.AP(
                            tensor=cst.tensor,
                            offset=cst.offset + D * 192,
                            ap=[[192, parts], [0, cols // 128], [1, 128]],
                        )
                    ),
                )

            kTs = [
                [persist.tile([128, SB], F32, name=f"kT{h}_{cc}") for cc in range(nsb)]
                for h in range(HL)
            ]
            for h in range(HL):
                for cc in range(nsb):
                    dma_zero(kTs[h][cc][D:128, :], D, SB)
            v_ts = [
                persist.tile([128, 4, HL, D + 1], F32, name=f"v_{cc}")
                for cc in range(nsb)
            ]

            import contextlib as _cl

            loop_cm = tc.For_i(0, repeat, 1) if repeat > 1 else _cl.nullcontext()

            def phase1_gen(cn, qS_out):
                # q^T/k^T/v for tokens [cn*SB, (cn+1)*SB), yielding between
                # independently schedulable pieces
                xT = xtpool.tile([128, NEB, SB], F32, tag="xT")
                nc.sync.dma_start(
                    out=_mm(xT),
                    in_=_mm(
                        bass.AP(
                            tensor=xbT.tensor,
                            offset=xbT.offset + cn * SB,
                            ap=[[t_len, 128], [128 * t_len, NEB], [1, SB]],
                        )
                    ),
                )
                yield
                for h in range(HL):
                    ps_qk = psA.tile([128, SB], F32, tag="psA", name="ps_qk")
                    for eb in range(NEB):
                        nc.tensor.matmul(
                            ps_qk,
                            lhsT=_mm(wqk_sb[:, eb, h * 128 : (h + 1) * 128]),
                            rhs=_mm(xT[:, eb, :]),
                            start=(eb == 0),
                            stop=(eb == NEB - 1),
                        )
                    # q padded to 128 partitions: rows 64.. multiply the zero
                    # rows of kT but must not contain inf/nan garbage
                    q_h = qspool.tile([128, SB], F32, tag="qS", name="q_h")
                    dma_zero(q_h[D:128, :], D, SB)
                    nc.vector.tensor_scalar_add(
                        out=_mm(q_h[0:D, :]),
                        in0=ps_qk[0:D, :],
                        scalar1=bqk_sb[:, h, 0:1],
                    )
                    qS_out.append(q_h)
                    # k rows live at psum partitions 64..127. Lane engines
                    # cannot shift partitions, so stage at the same partitions
                    # (adding bias) and let an SBUF->SBUF DMA move them to
                    # partition base 0 in kT.
                    kst = qspool.tile([128, SB], F32, tag="kst", name="kst", bufs=2)
                    nc.vector.tensor_scalar_add(
                        out=_mm(kst[D : 2 * D, :]),
                        in0=ps_qk[D : 2 * D, :],
                        scalar1=bk64_sb[D : 2 * D, h : h + 1],
                    )
                    nc.sync.dma_start(
                        out=_mm(kTs[h][cn][0:D, :]),
                        in_=_mm(kst[D : 2 * D, :]),
                    )
                    yield
                for tb in range(4):
                    ps_v = psA.tile([128, 256], F32, tag="psA", name="ps_v")
                    for eb in range(NEB):
                        nc.tensor.matmul(
                            ps_v,
                            lhsT=_mm(xT[:, eb, tb * 128 : (tb + 1) * 128]),
                            rhs=_mm(wv_sb[:, eb, :]),
                            start=(eb == 0),
                            stop=(eb == NEB - 1),
                        )
                    nc.vector.tensor_add(
                        out=_mm(v_ts[cn][:, tb, :, 0:D]),
                        in0=ps_v[:, 0:CH].rearrange("p (h d) -> p h d", h=HL),
                        in1=bv_bc[:, 0:CH].rearrange("p (h d) -> p h d", h=HL),
                    )
                    # ones column of v_aug: psum cols CH..CH+2 are x @ 0 = 0,
                    # plus the ones carried in the padded bias
                    nc.vector.tensor_add(
                        out=_mm(v_ts[cn][:, tb, :, D : D + 1]),
                        in0=ps_v[:, CH : CH + HL].rearrange(
                            "p (h o) -> p h o", o=1
                        ),
                        in1=bv_bc[:, CH : CH + HL].rearrange(
                            "p (h o) -> p h o", o=1
                        ),
                    )
                    yield

            def phase3_gen(cp, oS):
                # out-projection for t-blocks of superblock cp. ps_y borrows
                # the psA tag: a dedicated psum tag would let a stalled ps_y
                # alloc head-of-line-block the attention stream behind it.
                oS01p, oS2p = oS
                for tb in range(4):
                    tg = cp * 4 + tb
                    y_sb = ypool.tile([128, E], F32, tag="y_sb", name="y_sb")
                    for half in range(2):
                        ps_y = psA.tile([128, 384], F32, tag="psA", name="ps_y")
                        nc.tensor.matmul(
                            ps_y,
                            lhsT=_mm(oS01p[:, tb * KB : (tb + 1) * KB]),
                            rhs=_mm(wo01_sb[:, half * 384 : (half + 1) * 384]),
                            start=True,
                            stop=False,
                        )
                        nc.tensor.matmul(
                            ps_y,
                            lhsT=_mm(oS2p[:, tb * KB : (tb + 1) * KB]),
                            rhs=_mm(wo2_sb[:, half * 384 : (half + 1) * 384]),
                            start=False,
                            stop=True,
                        )
                        nc.vector.tensor_copy(
                            out=y_sb[:, half * 384 : (half + 1) * 384], in_=ps_y
                        )
                    nc.sync.dma_start(
                        out=y[tg * KB : (tg + 1) * KB, :], in_=y_sb
                    )
                    yield

            with loop_cm:
              qS = []
              for _ in phase1_gen(0, qS):
                  pass
              for c in range(nsb):
                nj = 4 * c + 4
                npair = nj // 2

                # phase-1 of chunk c+1 and phase-3 of chunk c-1 are emitted
                # piecewise between the attention pairs of chunk c: they keep
                # the PE busy while the ACT engine works through the exps.
                pieces = []
                qS_next = []
                if c + 1 < nsb:
                    pieces.append(phase1_gen(c + 1, qS_next))
                if c > 0:
                    pieces.append(phase3_gen(c - 1, oS_prev))
                stride = max(1, (3 * npair) // 14)
                pair_ctr = [0]

                def drive():
                    pair_ctr[0] += 1
                    if pair_ctr[0] % stride:
                        return
                    while pieces:
                        try:
                            next(pieces[0])
                            return
                        except StopIteration:
                            pieces.pop(0)

                # ======== phase 2: attention superblock i=c, all local heads
                oS01 = ospool.tile([128, SB], F32, tag="oS01", name="oS01")
                oS2 = ospool.tile([128, SB], F32, tag="oS2", name="oS2")
                dma_zero(oS2[D:128, :], D, SB)
                oS_prev_local = (oS01, oS2)

                def q0_of(j):
                    # causal slice: key block j only sees queries
                    # >= j*KB - c*SB; keep the moving dim >= 256 so
                    # fp32r stays at full rate
                    if j < 4 * c:
                        return 0
                    return min((j - 4 * c) * KB, SB - 256)

                def norm_chain(h, ps_o):
                    # PV(h) -> DVE recip -> PE bcast -> DVE mul
                    recip = rpool.tile([65, SB], F32, tag="recip", name="recip")
                    nc.vector.reciprocal(_mm(recip[64:65, :]), ps_o[D : D + 1, :])
                    ps_b = psA.tile([128, SB], F32, tag="psA", name="ps_b")
                    nc.tensor.matmul(
                        ps_b[0:D, :],
                        lhsT=_mm(ones65[64:65, :]),
                        rhs=_mm(recip[64:65, :]),
                        start=True,
                        stop=True,
                    )
                    # walrus: a DVE op may read only ONE non-scalar PSUM
                    # input, so stage the broadcast row in SBUF
                    rb = rpool.tile([D, SB], F32, tag="rbcast", name="rb")
                    nc.vector.tensor_copy(out=rb, in_=ps_b[0:D, :])
                    if h == 0:
                        o_dst = oS01[0:D, :]
                    elif h == 2:
                        o_dst = oS2[0:D, :]
                    else:
                        o_dst = ospool.tile([D, SB], F32, tag="o1tmp", name="o1tmp")
                    nc.vector.tensor_mul(_mm(o_dst), ps_o[0:D, :], rb)
                    if h == 1:
                        # stack h1 under h0 (partitions 64:128) via DMA, the
                        # only engine that can shift partitions
                        nc.sync.dma_start(
                            out=_mm(oS01[D : 2 * D, :]), in_=_mm(o_dst)
                        )

                def stream(h, depth=2):
                    ps_o = psO.tile([128, SB], F32, tag="psO", name="ps_o")
                    q_ap = qS[h]
                    pend = []

                    def pv_step(j, q0, pt_ap):
                        nc.tensor.matmul(
                            ps_o[0 : D + 1, q0:],
                            lhsT=_mm(v_ts[j // 4][:, j % 4, h, :]),
                            rhs=_mm(pt_ap[:, q0:]),
                            start=(j == 0),
                            stop=(j == nj - 1),
                        )

                    for jp in range(npair):
                        j0, j1 = 2 * jp, 2 * jp + 1
                        q00 = q0_of(j0)
                        ps_s2 = psS.tile([128, 2, SB], F32, tag="psS", name="ps_s2")
                        for half, j in ((0, j0), (1, j1)):
                            nc.tensor.matmul(
                                ps_s2[:, half, q00:],
                                lhsT=_mm(kTs[h][j // 4][:, (j % 4) * KB : (j % 4 + 1) * KB]),
                                rhs=_mm(q_ap[:, q00:]),
                                start=True,
                                stop=True,
                            )
                        pt2 = ptpool.tile([128, 2, SB], F32, tag="pt", name="pt2")
                        nc.scalar.activation(
                            out=_mm(pt2[:, :, q00:]),
                            in_=ps_s2[:, :, q00:],
                            func=mybir.ActivationFunctionType.Exp,
                            scale=float(SCALE),
                        )
                        for half, j in ((0, j0), (1, j1)):
                            if j >= 4 * c:
                                nc.gpsimd.affine_select(
                                    out=_mm(pt2[:, half, q00:]),
                                    in_=_mm(pt2[:, half, q00:]),
                                    compare_op=mybir.AluOpType.is_ge,
                                    fill=0.0,
                                    base=c * SB - j * KB + q00,
                                    pattern=[[1, SB - q00]],
                                    channel_multiplier=-1,
                                )
                        pend.append((j0, q00, pt2[:, 0, :]))
                        pend.append((j1, q00, pt2[:, 1, :]))
                        while len(pend) > 2 * depth:
                            pv_step(*pend.pop(0))
                        drive()
                    for jq in pend:
                        pv_step(*jq)
                    return ps_o

                prev = None
                for h in range(HL):
                    ps_o_h = stream(h)
                    if prev is not None:
                        norm_chain(*prev)
                    prev = (h, ps_o_h)
                norm_chain(*prev)
                while pieces:
                    try:
                        next(pieces[0])
                    except StopIteration:
                        pieces.pop(0)
                oS_prev = oS_prev_local
                qS = qS_next
              for _ in phase3_gen(nsb - 1, oS_prev):
                  pass
    nc.compile()
    return nc


def make_in_maps(x, wq, bq, wk, bk, wv, bv, wo, bo, t_len=T):
    x = np.asarray(x, np.float32)
    in_maps = []
    for c in range(8):
        b, g = divmod(c, 4)
        hs = slice(g * CH, (g + 1) * CH)
        wqk_c = np.empty((E, 2 * CH), np.float32)
        bqk_c = np.empty((HL, 2, D), np.float32)
        for hl in range(HL):
            h = g * HL + hl
            wqk_c[:, hl * 128 : hl * 128 + D] = wq[:, h * D : (h + 1) * D]
            wqk_c[:, hl * 128 + D : (hl + 1) * 128] = wk[:, h * D : (h + 1) * D]
            bqk_c[hl, 0] = bq[h * D : (h + 1) * D]
            bqk_c[hl, 1] = bk[h * D : (h + 1) * D]
        wv_c = np.zeros((E, 256), np.float32)
        wv_c[:, :CH] = wv[:, hs]
        bv_c = np.ones(CH + HL, np.float32)
        bv_c[:CH] = np.asarray(bv, np.float32)[hs]
        wo_c = np.zeros((256, E), np.float32)
        wo_c[:CH] = np.asarray(wo, np.float32)[hs]
        # cols 0..127: zeros (device-side zero fills); cols 128..191: ones
        cst = np.concatenate(
            [np.zeros((128, 128), np.float32), np.ones((128, 64), np.float32)],
            axis=1,
        )
        in_maps.append(
            {
                "xbT": np.ascontiguousarray(x[b, :t_len].T),
                "wqk": wqk_c,
                "wvp": wv_c,
                "wo": wo_c,
                "bqk": bqk_c,
                "bv": bv_c,
                "cst": cst,
            }
        )
    return in_maps


_NC_CACHE = {}


def get_nc(t_len=T):
    if t_len not in _NC_CACHE:
        _NC_CACHE[t_len] = build_nc(t_len)
    return _NC_CACHE[t_len]


def _build_sharded_nodonate(nc, n_cores=8):
    """Mirror bass2jax.run_bass_via_pjrt's multi-core path, minus donation,
    returning (jitted_fn, in_names, out_names, out_avals). Without donation a
    call can be repeated on device-resident arrays for timing. Safe here: the
    kernel writes every element of y."""
    import jax
    from jax.sharding import Mesh, PartitionSpec
    from jax.experimental.shard_map import shard_map

    from concourse import bass2jax
    from concourse.bass2jax import _bass_exec_p

    bass2jax.install_neuronx_cc_hook()
    part_name = nc.partition_id_tensor.name if nc.partition_id_tensor else None

    in_names, out_names, out_avals = [], [], []
    for alloc in nc.m.functions[0].allocations:
        if not isinstance(alloc, mybir.MemoryLocationSet):
            continue
        name = alloc.memorylocations[0].name
        if alloc.kind == "ExternalInput":
            if name != part_name:
                in_names.append(name)
        elif alloc.kind == "ExternalOutput":
            shape = tuple(alloc.tensor_shape)
            dtype = mybir.dt.np(alloc.dtype)
            out_names.append(name)
            out_avals.append(jax.core.ShapedArray(shape, dtype))
    n_params = len(in_names)
    all_names = in_names + out_names
    if part_name is not None:
        all_names = all_names + [part_name]

    def _body(*args):
        operands = list(args)
        if part_name is not None:
            operands.append(bass2jax.partition_id_tensor())
        outs = _bass_exec_p.bind(
            *operands,
            out_avals=tuple(out_avals),
            in_names=tuple(all_names),
            out_names=tuple(out_names),
            lowering_input_output_aliases=(),
            sim_require_finite=True,
            sim_require_nnan=True,
            nc=nc,
        )
        return tuple(outs)

    devices = jax.devices()[:n_cores]
    mesh = Mesh(np.asarray(devices), ("core",))
    n_out = len(out_names)
    sharded = jax.jit(
        shard_map(
            _body,
            mesh=mesh,
            in_specs=(PartitionSpec("core"),) * (n_params + n_out),
            out_specs=(PartitionSpec("core"),) * n_out,
            check_rep=False,
        ),
        keep_unused=True,
    )
    return sharded, in_names, out_names, out_avals


def run_timed(nc, in_maps, iters=20):
    """Execute on HW repeatedly with device-resident args; returns
    (per-core results, sorted per-call walls in seconds)."""
    import time

    import jax

    n_cores = len(in_maps)
    sharded, in_names, out_names, out_avals = _build_sharded_nodonate(nc, n_cores)
    concat_in = [
        np.concatenate([np.asarray(m[name]) for m in in_maps], axis=0)
        for name in in_names
    ]
    concat_zero = [
        np.zeros((n_cores * a.shape[0], *a.shape[1:]), a.dtype) for a in out_avals
    ]
    args = [jax.device_put(a) for a in concat_in + concat_zero]
    out = sharded(*args)  # compile + first run
    jax.block_until_ready(out)
    walls = []
    for _ in range(iters):
        t0 = time.perf_counter()
        out2 = sharded(*args)
        jax.block_until_ready(out2)
        walls.append(time.perf_counter() - t0)
    results = [
        {
            name: np.asarray(out[i]).reshape(n_cores, *out_avals[i].shape)[c]
            for i, name in enumerate(out_names)
        }
        for c in range(n_cores)
    ]
    return results, sorted(walls)


def baseline_rtt(iters=20):
    """Axon dispatch floor: same path with a trivial 8-core kernel."""
    nc = bacc.Bacc("TRN2", target_bir_lowering=False, debug=False, num_devices=8)
    a = nc.dram_tensor("a", [128, 128], F32, kind="ExternalInput")
    b = nc.dram_tensor("b", [128, 128], F32, kind="ExternalOutput")
    a, b = a.ap(), b.ap()
    with tile.TileContext(nc) as tc:
        with tc.tile_pool(name="p", bufs=1) as p:
            t = p.tile([128, 128], F32)
            nc.sync.dma_start(out=t, in_=a)
            nc.scalar.mul(out=t, in_=t, mul=2.0)
            nc.sync.dma_start(out=b, in_=t)
    nc.compile()
    in_maps = [{"a": np.zeros((128, 128), np.float32)} for _ in range(8)]
    _, walls = run_timed(nc, in_maps, iters=iters)
    return walls


def kernel(x, wq, bq, wk, bk, wv, bv, wo, bo, _trace=False, _trace_kwargs=None):
    nc = get_nc()
    in_maps = make_in_maps(x, wq, bq, wk, bk, wv, bv, wo, bo)
    res = run_bass_kernel_spmd(
        nc, in_maps, list(range(8)), trace=_trace, **(_trace_kwargs or {})
    )
    bo = np.asarray(bo, np.float32)
    out = np.empty((B, T, E), np.float32)
    for b in range(B):
        acc = res.results[b * 4]["y"].astype(np.float32).copy()
        for g in range(1, 4):
            acc += res.results[b * 4 + g]["y"]
        out[b] = acc + bo
    if _trace:
        return out, res
    return out


# revision 17
# speedup vs baseline: 1.6519x; 1.2566x over previous
"""Causal self-attention Trainium2 kernel (B=2, T=4096, E=768, H=12, D=64).

Sharding: 8 cores = 2 batches x 4 head-groups (3 heads each). Each core:
  - receives x pre-transposed from host (xbT [E, T]) so no PE transposes,
  - computes q/k in transposed layout [d, t] and v in natural layout [t, d]
    for its 3 heads (fp32r matmuls),
  - causal attention in S^T layout ([key, query] tiles). All attention
    matmul contractions are padded to K=128 (kT rows 64..127 are zero);
    K=64 matmuls run ~2.7x slower on HW (measured) than K=128,
  - exp on ACT per key-block pair, denominator via an extra ones-column
    appended to v (PV matmul row 64 = sum of exp),
  - normalizes via reciprocal + PE broadcast, out-projects with its wo
    row-slice (zero-padded to 128 rows for the second K slice) producing a
    partial y [4096, 768].
Host sums the 4 partials per batch and adds bo.
"""

import os
import sys

sys.path.insert(0, "/opt/trn_rl_repo")

import numpy as np

try:  # persistent jit cache: skips the ~10min neuronxcc compile on re-runs
    import jax

    jax.config.update("jax_compilation_cache_dir", "/tmp/jax_neff_cache")
    jax.config.update("jax_persistent_cache_min_compile_time_secs", 10)
    jax.config.update("jax_persistent_cache_min_entry_size_bytes", 0)
except Exception:
    pass

import concourse.bass as bass
import concourse.mybir as mybir
import concourse.tile as tile
from concourse import bacc
from concourse.bass_utils import run_bass_kernel_spmd

F32 = mybir.dt.float32
F32R = mybir.dt.float32r

B, T, E, H = 2, 4096, 768, 12
D = E // H            # 64
HL = 3                # heads per core
CH = HL * D           # 192 channels per core
SB = 512              # query superblock
KB = 128              # key block
NEB = E // 128        # 6 embed tiles
SCALE = 1.0 / np.sqrt(D)


def _mm(ap):
    return ap.bitcast(F32R) if ap.dtype == F32 else ap


def build_nc(t_len=T, repeat=1):
    assert t_len % SB == 0
    nsb = t_len // SB       # superblocks
    ntb = t_len // KB       # 128-blocks

    nc = bacc.Bacc("TRN2", target_bir_lowering=False, debug=False, num_devices=8)

    xbT = nc.dram_tensor("xbT", [E, t_len], F32, kind="ExternalInput")
    wqk = nc.dram_tensor("wqk", [E, 2 * CH], F32, kind="ExternalInput")
    wvp = nc.dram_tensor("wvp", [E, 256], F32, kind="ExternalInput")
    wo = nc.dram_tensor("wo", [256, E], F32, kind="ExternalInput")
    bqk = nc.dram_tensor("bqk", [HL, 2, D], F32, kind="ExternalInput")
    bv = nc.dram_tensor("bv", [CH + HL], F32, kind="ExternalInput")
    cst = nc.dram_tensor("cst", [128, 192], F32, kind="ExternalInput")
    y = nc.dram_tensor("y", [t_len, E], F32, kind="ExternalOutput")

    xbT, wqk, wvp, wo, bqk, bv, cst, y = (
        t.ap() for t in (xbT, wqk, wvp, wo, bqk, bv, cst, y)
    )

    with tile.TileContext(nc) as tc:
        import contextlib

        ctx = contextlib.ExitStack()
        with ctx:
            ctx.enter_context(
                nc.allow_low_precision(reason="fp32r rounding of matmul operands")
            )
            const = ctx.enter_context(tc.tile_pool(name="const", bufs=1))
            persist = ctx.enter_context(tc.tile_pool(name="persist", bufs=1))
            xtpool = ctx.enter_context(tc.tile_pool(name="xtpool", bufs=2))
            qspool = ctx.enter_context(tc.tile_pool(name="qspool", bufs=6))
            ospool = ctx.enter_context(tc.tile_pool(name="ospool", bufs=2))
            ptpool = ctx.enter_context(tc.tile_pool(name="ptpool", bufs=4))
            rpool = ctx.enter_context(tc.tile_pool(name="rpool", bufs=2))
            ypool = ctx.enter_context(tc.tile_pool(name="ypool", bufs=2))
            psA = ctx.enter_context(tc.tile_pool(name="psA", bufs=2, space="PSUM"))
            psS = ctx.enter_context(tc.tile_pool(name="psS", bufs=2, space="PSUM"))
            psO = ctx.enter_context(tc.tile_pool(name="psO", bufs=2, space="PSUM"))

            # ---- constants / weights in SBUF ----
            ones65 = const.tile([65, D], F32)
            nc.sync.dma_start(
                out=_mm(ones65[64:65, :]), in_=_mm(cst[64:65, 128 : 128 + D])
            )

            wqk_sb = const.tile([128, NEB, 2 * CH], F32)
            nc.sync.dma_start(
                out=_mm(wqk_sb), in_=_mm(wqk).rearrange("(n p) m -> p n m", p=128)
            )
            wv_sb = const.tile([128, NEB, 256], F32)
            nc.sync.dma_start(
                out=_mm(wv_sb), in_=_mm(wvp).rearrange("(n p) m -> p n m", p=128)
            )
            wo01_sb = const.tile([128, E], F32)
            nc.sync.dma_start(out=_mm(wo01_sb), in_=_mm(wo[0:128, :]))
            # rows 64..127 of the second K slice are zero (host-padded)
            wo2_sb = const.tile([128, E], F32)
            nc.sync.dma_start(out=_mm(wo2_sb), in_=_mm(wo[128:256, :]))
            bqk_sb = const.tile([D, HL, 2], F32)
            nc.sync.dma_start(out=bqk_sb, in_=bqk.rearrange("h q p -> p h q"))
            # k-bias copy living at partitions 64..127 (k rows of the packed
            # qk psum) so the staging add is partition-aligned
            bk64_sb = const.tile([128, HL], F32)
            nc.sync.dma_start(
                out=bk64_sb[D : 2 * D, :], in_=bqk[:, 1, :].rearrange("h p -> p h")
            )
            bv_bc = const.tile([128, CH + HL], F32)
            nc.sync.dma_start(
                out=bv_bc,
                in_=bass.AP(
                    tensor=bv.tensor, offset=bv.offset, ap=[[0, 128]] + list(bv.ap)
                ),
            )

            # persistent activations, split per superblock chunk so the
            # interleaved phase-1 writes of chunk c+1 never alias the
            # attention reads of chunk c. kT rows 64..127 stay zero: the
            # K=128 S-matmul contraction is padded (K=64 matmuls run ~2.7x
            # slower on HW), and the padded q rows must multiply zeros.
            def dma_zero(dst, parts, cols):
                # cst cols 0..127 are zeros (host); stride-0 repeat covers cols
                assert cols % 128 == 0
                nc.sync.dma_start(
                    out=_mm(dst),
                    in_=_mm(
                        bass.AP(
                            tensor=cst.tensor,
                            offset=cst.offset + D * 192,
                            ap=[[192, parts], [0, cols // 128], [1, 128]],
                        )
                    ),
                )

            kTs = [
                [persist.tile([128, SB], F32, name=f"kT{h}_{cc}") for cc in range(nsb)]
                for h in range(HL)
            ]
            for h in range(HL):
                for cc in range(nsb):
                    dma_zero(kTs[h][cc][D:128, :], D, SB)
            v_ts = [
                persist.tile([128, 4, HL, D + 1], F32, name=f"v_{cc}")
                for cc in range(nsb)
            ]

            import contextlib as _cl

            loop_cm = tc.For_i(0, repeat, 1) if repeat > 1 else _cl.nullcontext()

            def phase1_gen(cn, qS_out):
                # q^T/k^T/v for tokens [cn*SB, (cn+1)*SB), yielding between
                # independently schedulable pieces
                xT = xtpool.tile([128, NEB, SB], F32, tag="xT")
                nc.sync.dma_start(
                    out=_mm(xT),
                    in_=_mm(
                        bass.AP(
                            tensor=xbT.tensor,
                            offset=xbT.offset + cn * SB,
                            ap=[[t_len, 128], [128 * t_len, NEB], [1, SB]],
                        )
                    ),
                )
                yield
                for h in range(HL):
                    ps_qk = psA.tile([128, SB], F32, tag="psA", name="ps_qk")
                    for eb in range(NEB):
                        nc.tensor.matmul(
                            ps_qk,
                            lhsT=_mm(wqk_sb[:, eb, h * 128 : (h + 1) * 128]),
                            rhs=_mm(xT[:, eb, :]),
                            start=(eb == 0),
                            stop=(eb == NEB - 1),
                        )
                    # q padded to 128 partitions: rows 64.. multiply the zero
                    # rows of kT but must not contain inf/nan garbage
                    q_h = qspool.tile([128, SB], F32, tag="qS", name="q_h")
                    dma_zero(q_h[D:128, :], D, SB)
                    nc.vector.tensor_scalar_add(
                        out=_mm(q_h[0:D, :]),
                        in0=ps_qk[0:D, :],
                        scalar1=bqk_sb[:, h, 0:1],
                    )
                    qS_out.append(q_h)
                    # k rows live at psum partitions 64..127. Lane engines
                    # cannot shift partitions, so stage at the same partitions
                    # (adding bias) and let an SBUF->SBUF DMA move them to
                    # partition base 0 in kT.
                    kst = qspool.tile([128, SB], F32, tag="kst", name="kst", bufs=2)
                    nc.vector.tensor_scalar_add(
                        out=_mm(kst[D : 2 * D, :]),
                        in0=ps_qk[D : 2 * D, :],
                        scalar1=bk64_sb[D : 2 * D, h : h + 1],
                    )
                    nc.sync.dma_start(
                        out=_mm(kTs[h][cn][0:D, :]),
                        in_=_mm(kst[D : 2 * D, :]),
                    )
                    yield
                for tb in range(4):
                    ps_v = psA.tile([128, 256], F32, tag="psA", name="ps_v")
                    for eb in range(NEB):
                        nc.tensor.matmul(
                            ps_v,
                            lhsT=_mm(xT[:, eb, tb * 128 : (tb + 1) * 128]),
                            rhs=_mm(wv_sb[:, eb, :]),
                            start=(eb == 0),
                            stop=(eb == NEB - 1),
                        )
                    nc.vector.tensor_add(
                        out=_mm(v_ts[cn][:, tb, :, 0:D]),
                        in0=ps_v[:, 0:CH].rearrange("p (h d) -> p h d", h=HL),
                        in1=bv_bc[:, 0:CH].rearrange("p (h d) -> p h d", h=HL),
                    )
                    # ones column of v_aug: psum cols CH..CH+2 are x @ 0 = 0,
                    # plus the ones carried in the padded bias
                    nc.vector.tensor_add(
                        out=_mm(v_ts[cn][:, tb, :, D : D + 1]),
                        in0=ps_v[:, CH : CH + HL].rearrange(
                            "p (h o) -> p h o", o=1
                        ),
                        in1=bv_bc[:, CH : CH + HL].rearrange(
                            "p (h o) -> p h o", o=1
                        ),
                    )
                    yield

            def phase3_gen(cp, oS):
                # out-projection for t-blocks of superblock cp. ps_y borrows
                # the psA tag: a dedicated psum tag would let a stalled ps_y
                # alloc head-of-line-block the attention stream behind it.
                oS01p, oS2p = oS
                for tb in range(4):
                    tg = cp * 4 + tb
                    y_sb = ypool.tile([128, E], F32, tag="y_sb", name="y_sb")
                    for half in range(2):
                        ps_y = psA.tile([128, 384], F32, tag="psA", name="ps_y")
                        nc.tensor.matmul(
                            ps_y,
                            lhsT=_mm(oS01p[:, tb * KB : (tb + 1) * KB]),
                            rhs=_mm(wo01_sb[:, half * 384 : (half + 1) * 384]),
                            start=True,
                            stop=False,
                        )
                        nc.tensor.matmul(
                            ps_y,
                            lhsT=_mm(oS2p[:, tb * KB : (tb + 1) * KB]),
                            rhs=_mm(wo2_sb[:, half * 384 : (half + 1) * 384]),
                            start=False,
                            stop=True,
                        )
                        nc.vector.tensor_copy(
                            out=y_sb[:, half * 384 : (half + 1) * 384], in_=ps_y
                        )
                    nc.sync.dma_start(
                        out=y[tg * KB : (tg + 1) * KB, :], in_=y_sb
                    )
                    yield

            with loop_cm:
              qS = []
              for _ in phase1_gen(0, qS):
                  pass
              for c in range(nsb):
                nj = 4 * c + 4
                npair = nj // 2

                # phase-1 of chunk c+1 and phase-3 of chunk c-1 are emitted
                # piecewise between the attention pairs of chunk c: they keep
                # the PE busy while the ACT engine works through the exps.
                pieces = []
                qS_next = []
                if c + 1 < nsb:
                    pieces.append(phase1_gen(c + 1, qS_next))
                if c > 0:
                    pieces.append(phase3_gen(c - 1, oS_prev))
                stride = max(1, (3 * npair) // 14)
                pair_ctr = [0]

                def drive():
                    pair_ctr[0] += 1
                    if pair_ctr[0] % stride:
                        return
                    while pieces:
                        try:
                            next(pieces[0])
                            return
                        except StopIteration:
                            pieces.pop(0)

                # ======== phase 2: attention superblock i=c, all local heads
                oS01 = ospool.tile([128, SB], F32, tag="oS01", name="oS01")
                oS2 = ospool.tile([128, SB], F32, tag="oS2", name="oS2")
                dma_zero(oS2[D:128, :], D, SB)
                oS_prev_local = (oS01, oS2)

                def q0_of(j):
                    # causal slice: key block j only sees queries
                    # >= j*KB - c*SB; keep the moving dim >= 256 so
                    # fp32r stays at full rate
                    if j < 4 * c:
                        return 0
                    return min((j - 4 * c) * KB, SB - 256)

                def norm_chain(h, ps_o):
                    # PV(h) -> DVE recip -> PE bcast -> DVE mul
                    recip = rpool.tile([65, SB], F32, tag="recip", name="recip")
                    nc.vector.reciprocal(_mm(recip[64:65, :]), ps_o[D : D + 1, :])
                    ps_b = psA.tile([128, SB], F32, tag="psA", name="ps_b")
                    nc.tensor.matmul(
                        ps_b[0:D, :],
                        lhsT=_mm(ones65[64:65, :]),
                        rhs=_mm(recip[64:65, :]),
                        start=True,
                        stop=True,
                    )
                    # walrus: a DVE op may read only ONE non-scalar PSUM
                    # input, so stage the broadcast row in SBUF
                    rb = rpool.tile([D, SB], F32, tag="rbcast", name="rb")
                    nc.vector.tensor_copy(out=rb, in_=ps_b[0:D, :])
                    if h == 0:
                        o_dst = oS01[0:D, :]
                    elif h == 2:
                        o_dst = oS2[0:D, :]
                    else:
                        o_dst = ospool.tile([D, SB], F32, tag="o1tmp", name="o1tmp")
                    nc.vector.tensor_mul(_mm(o_dst), ps_o[0:D, :], rb)
                    if h == 1:
                        # stack h1 under h0 (partitions 64:128) via DMA, the
                        # only engine that can shift partitions
                        nc.sync.dma_start(
                            out=_mm(oS01[D : 2 * D, :]), in_=_mm(o_dst)
                        )

                def stream(h, depth=3):
                    ps_o = psO.tile([128, SB], F32, tag="psO", name="ps_o")
                    q_ap = qS[h]
                    pend = []

                    def pv_step(j, q0, pt_ap):
                        nc.tensor.matmul(
                            ps_o[0 : D + 1, q0:],
                            lhsT=_mm(v_ts[j // 4][:, j % 4, h, :]),
                            rhs=_mm(pt_ap[:, q0:]),
                            start=(j == 0),
                            stop=(j == nj - 1),
                        )

                    for jp in range(npair):
                        j0, j1 = 2 * jp, 2 * jp + 1
                        q00 = q0_of(j0)
                        ps_s2 = psS.tile([128, 2, SB], F32, tag="psS", name="ps_s2")
                        for half, j in ((0, j0), (1, j1)):
                            nc.tensor.matmul(
                                ps_s2[:, half, q00:],
                                lhsT=_mm(kTs[h][j // 4][:, (j % 4) * KB : (j % 4 + 1) * KB]),
                                rhs=_mm(q_ap[:, q00:]),
                                start=True,
                                stop=True,
                            )
                        pt2 = ptpool.tile([128, 2, SB], F32, tag="pt", name="pt2")
                        nc.scalar.activation(
                            out=_mm(pt2[:, :, q00:]),
                            in_=ps_s2[:, :, q00:],
                            func=mybir.ActivationFunctionType.Exp,
                            scale=float(SCALE),
                        )
                        for half, j in ((0, j0), (1, j1)):
                            if j >= 4 * c:
                                nc.gpsimd.affine_select(
                                    out=_mm(pt2[:, half, q00:]),
                                    in_=_mm(pt2[:, half, q00:]),
                                    compare_op=mybir.AluOpType.is_ge,
                                    fill=0.0,
                                    base=c * SB - j * KB + q00,
                                    pattern=[[1, SB - q00]],
                                    channel_multiplier=-1,
                                )
                        pend.append((j0, q00, pt2[:, 0, :]))
                        pend.append((j1, q00, pt2[:, 1, :]))
                        while len(pend) > 2 * depth:
                            pv_step(*pend.pop(0))
                        drive()
                    for jq in pend:
                        pv_step(*jq)
                    return ps_o

                prev = None
                for h in range(HL):
                    ps_o_h = stream(h)
                    if prev is not None:
                        norm_chain(*prev)
                    prev = (h, ps_o_h)
                norm_chain(*prev)
                while pieces:
                    try:
                        next(pieces[0])
                    except StopIteration:
                        pieces.pop(0)
                oS_prev = oS_prev_local
                qS = qS_next
              for _ in phase3_gen(nsb - 1, oS_prev):
                  pass
    nc.compile()
    return nc


def make_in_maps(x, wq, bq, wk, bk, wv, bv, wo, bo, t_len=T):
    x = np.asarray(x, np.float32)
    in_maps = []
    for c in range(8):
        b, g = divmod(c, 4)
        hs = slice(g * CH, (g + 1) * CH)
        wqk_c = np.empty((E, 2 * CH), np.float32)
        bqk_c = np.empty((HL, 2, D), np.float32)
        for hl in range(HL):
            h = g * HL + hl
            wqk_c[:, hl * 128 : hl * 128 + D] = wq[:, h * D : (h + 1) * D]
            wqk_c[:, hl * 128 + D : (hl + 1) * 128] = wk[:, h * D : (h + 1) * D]
            bqk_c[hl, 0] = bq[h * D : (h + 1) * D]
            bqk_c[hl, 1] = bk[h * D : (h + 1) * D]
        wv_c = np.zeros((E, 256), np.float32)
        wv_c[:, :CH] = wv[:, hs]
        bv_c = np.ones(CH + HL, np.float32)
        bv_c[:CH] = np.asarray(bv, np.float32)[hs]
        wo_c = np.zeros((256, E), np.float32)
        wo_c[:CH] = np.asarray(wo, np.float32)[hs]
        # cols 0..127: zeros (device-side zero fills); cols 128..191: ones
        cst = np.concatenate(
            [np.zeros((128, 128), np.float32), np.ones((128, 64), np.float32)],
            axis=1,
        )
        in_maps.append(
            {
                "xbT": np.ascontiguousarray(x[b, :t_len].T),
                "wqk": wqk_c,
                "wvp": wv_c,
                "wo": wo_c,
                "bqk": bqk_c,
                "bv": bv_c,
                "cst": cst,
            }
        )
    return in_maps


_NC_CACHE = {}


def get_nc(t_len=T):
    if t_len not in _NC_CACHE:
        _NC_CACHE[t_len] = build_nc(t_len)
    return _NC_CACHE[t_len]


def _build_sharded_nodonate(nc, n_cores=8):
    """Mirror bass2jax.run_bass_via_pjrt's multi-core path, minus donation,
    returning (jitted_fn, in_names, out_names, out_avals). Without donation a
    call can be repeated on device-resident arrays for timing. Safe here: the
    kernel writes every element of y."""
    import jax
    from jax.sharding import Mesh, PartitionSpec
    from jax.experimental.shard_map import shard_map

    from concourse import bass2jax
    from concourse.bass2jax import _bass_exec_p

    bass2jax.install_neuronx_cc_hook()
    part_name = nc.partition_id_tensor.name if nc.partition_id_tensor else None

    in_names, out_names, out_avals = [], [], []
    for alloc in nc.m.functions[0].allocations:
        if not isinstance(alloc, mybir.MemoryLocationSet):
            continue
        name = alloc.memorylocations[0].name
        if alloc.kind == "ExternalInput":
            if name != part_name:
                in_names.append(name)
        elif alloc.kind == "ExternalOutput":
            shape = tuple(alloc.tensor_shape)
            dtype = mybir.dt.np(alloc.dtype)
            out_names.append(name)
            out_avals.append(jax.core.ShapedArray(shape, dtype))
    n_params = len(in_names)
    all_names = in_names + out_names
    if part_name is not None:
        all_names = all_names + [part_name]

    def _body(*args):
        operands = list(args)
        if part_name is not None:
            operands.append(bass2jax.partition_id_tensor())
        outs = _bass_exec_p.bind(
            *operands,
            out_avals=tuple(out_avals),
            in_names=tuple(all_names),
            out_names=tuple(out_names),
            lowering_input_output_aliases=(),
            sim_require_finite=True,
            sim_require_nnan=True,
            nc=nc,
        )
        return tuple(outs)

    devices = jax.devices()[:n_cores]
    mesh = Mesh(np.asarray(devices), ("core",))
    n_out = len(out_names)
    sharded = jax.jit(
        shard_map(
            _body,
            mesh=mesh,
            in_specs=(PartitionSpec("core"),) * (n_params + n_out),
            out_specs=(PartitionSpec("core"),) * n_out,
            check_rep=False,
        ),
        keep_unused=True,
    )
    return sharded, in_names, out_names, out_avals


def run_timed(nc, in_maps, iters=20):
    """Execute on HW repeatedly with device-resident args; returns
    (per-core results, sorted per-call walls in seconds)."""
    import time

    import jax

    n_cores = len(in_maps)
    sharded, in_names, out_names, out_avals = _build_sharded_nodonate(nc, n_cores)
    concat_in = [
        np.concatenate([np.asarray(m[name]) for m in in_maps], axis=0)
        for name in in_names
    ]
    concat_zero = [
        np.zeros((n_cores * a.shape[0], *a.shape[1:]), a.dtype) for a in out_avals
    ]
    args = [jax.device_put(a) for a in concat_in + concat_zero]
    out = sharded(*args)  # compile + first run
    jax.block_until_ready(out)
    walls = []
    for _ in range(iters):
        t0 = time.perf_counter()
        out2 = sharded(*args)
        jax.block_until_ready(out2)
        walls.append(time.perf_counter() - t0)
    results = [
        {
            name: np.asarray(out[i]).reshape(n_cores, *out_avals[i].shape)[c]
            for i, name in enumerate(out_names)
        }
        for c in range(n_cores)
    ]
    return results, sorted(walls)


def baseline_rtt(iters=20):
    """Axon dispatch floor: same path with a trivial 8-core kernel."""
    nc = bacc.Bacc("TRN2", target_bir_lowering=False, debug=False, num_devices=8)
    a = nc.dram_tensor("a", [128, 128], F32, kind="ExternalInput")
    b = nc.dram_tensor("b", [128, 128], F32, kind="ExternalOutput")
    a, b = a.ap(), b.ap()
    with tile.TileContext(nc) as tc:
        with tc.tile_pool(name="p", bufs=1) as p:
            t = p.tile([128, 128], F32)
            nc.sync.dma_start(out=t, in_=a)
            nc.scalar.mul(out=t, in_=t, mul=2.0)
            nc.sync.dma_start(out=b, in_=t)
    nc.compile()
    in_maps = [{"a": np.zeros((128, 128), np.float32)} for _ in range(8)]
    _, walls = run_timed(nc, in_maps, iters=iters)
    return walls


def kernel(x, wq, bq, wk, bk, wv, bv, wo, bo, _trace=False, _trace_kwargs=None):
    nc = get_nc()
    in_maps = make_in_maps(x, wq, bq, wk, bk, wv, bv, wo, bo)
    res = run_bass_kernel_spmd(
        nc, in_maps, list(range(8)), trace=_trace, **(_trace_kwargs or {})
    )
    bo = np.asarray(bo, np.float32)
    out = np.empty((B, T, E), np.float32)
    for b in range(B):
        acc = res.results[b * 4]["y"].astype(np.float32).copy()
        for g in range(1, 4):
            acc += res.results[b * 4 + g]["y"]
        out[b] = acc + bo
    if _trace:
        return out, res
    return out
